# revision 19
# baseline (speedup 1.0000x reference)
"""IterNorm (training-mode whitening, num_groups=1) Bass/Tile kernel for 8 trn2 cores.

Strategy (data-parallel over batch B, per sharding hint):
  - Each of the 8 cores gets 4 of the 32 batches: X_shard (4, 64, 8192) f32.
  - Batches are stacked in pairs onto 128 SBUF partitions (p0-63 = even batch
    channels, 64-127 = odd batch channels); full 128-partition HBM DMAs.
  - Stats pass: per tile, f32 load -> DVE cast to fp16 shadow -> XBAR
    DMA-transpose (fp16 SBUF->SBUF on the ACT HWDGE queue, 14ns per 16x128
    tile) producing chunked (128, nch, 130) transposed layouts with a memset
    ones column -> accumulating 128x129 fp16 PE matmul with f32 PSUM
    (cols 0..127: stacked second-moment, col 128: channel sums).  No PE
    transposes, no PSUM->SBUF copies: PE does only the Gram matmuls and the
    stats phase is HBM-load-bound.
  - Stacked block folded locally to (64,65) (selector matmul), combined
    across cores with a 16.6 KB AllGather + on-chip reduction.
  - Replicated epilogue: Sigma via one fused DVE op (tot + M*outer)*(1/M),
    trace via fused diag-mask reduce + one broadcast matmul, 1 DVE
    reciprocal; Newton-Schulz in fp16 with iteration 1 folded into
    P1 = 1.5I - 0.5 Sigma_N (DVE add) and 4 PE iterations of
    {P2|Q paired matmuls in one PSUM bank -> one DVE copy -> C matmuls}.
  - Apply pass: mu pre-subtracted from the fp16 shadows in place (ACT and
    GpSimd engines, hidden under the NS iterations), then per (128,512)
    chunk: y = W2 @ xc on PE, PSUM->SBUF copy alternating DVE/ACT, per-chunk
    f32 stores.  W2 = blockdiag(wm, wm) fp16 via SBUF->SBUF DMA.

Self-contained: hardcodes shapes and builds all constant inputs on the host.
"""

import sys

for _p in ("/opt/trn_rl_repo",):
    if _p not in sys.path:
        sys.path.insert(0, _p)

import numpy as np

import concourse.bass as bass  # noqa: F401
import concourse.tile as tile
from concourse import bacc, mybir
from concourse.bass_utils import run_bass_kernel_spmd

NCORES = 8
B, C, L = 32, 64, 8192
BPC = B // NCORES            # batches per core
M_TOT = B * L
EPS = 1e-5                   # folded away: negligible vs fp16 stats noise
T_NS = 5
F32 = mybir.dt.float32
F16 = mybir.dt.float16
XT_PITCH = 130
XTILE_W = 2048
TILE_PLAN = [2048, 2048, 2048, 1536, 512]   # per pair; sums to L
TRANSPOSE_MODE = "dma_scalar"
CAST_MODE = "fused"
STOP_AFTER = "full"
EPI_VARIANT = "nottr"   # tensor_tensor_reduce crashes on hw

_CACHE = {}


def _build_bass(ncores=NCORES):
    nc = bacc.Bacc("TRN2", target_bir_lowering=False, debug=False, num_devices=ncores)

    X = nc.dram_tensor("X", [BPC, C, L], F32, kind="ExternalInput")
    Y = nc.dram_tensor("Y", [BPC, C, L], F32, kind="ExternalOutput")
    IDENT64 = nc.dram_tensor("IDENT64", [64, 64], F32, kind="ExternalInput")
    ESEL = nc.dram_tensor("ESEL", [128, 64], F32, kind="ExternalInput")
    F2H = nc.dram_tensor("F2H", [64, 128], F32, kind="ExternalInput")
    ONES64 = nc.dram_tensor("ONES64", [64, 64], F32, kind="ExternalInput")
    I15H = nc.dram_tensor("I15H", [64, 64], F16, kind="ExternalInput")
    IH16 = nc.dram_tensor("IH16", [64, 64], F16, kind="ExternalInput")
    ID128H = nc.dram_tensor("ID128H", [128, 128], F16, kind="ExternalInput")

    Xv = X.ap().rearrange("(p i) c l -> p (i c) l", i=2)
    Yv = Y.ap().rearrange("(p i) c l -> p (i c) l", i=2)
    tile_geom = []
    for pair in range(2):
        off = 0
        for w in TILE_PLAN:
            tile_geom.append((pair, off, w))
            off += w
    n_chunks = 2 * L // 128
    do_fold = STOP_AFTER in ("collective", "epilogue", "full")
    do_epi = STOP_AFTER in ("epilogue", "full")
    do_apply = STOP_AFTER == "full"

    with tile.TileContext(nc) as tc:
        with (
            tc.tile_pool(name="consts", bufs=1) as consts,
            tc.tile_pool(name="xpool", bufs=3) as xpool,
            tc.tile_pool(name="xTpool", bufs=3) as xTpool,
            tc.tile_pool(name="xbpool", bufs=1) as xbpool,
            tc.tile_pool(name="ypool", bufs=6) as ypool,
            tc.tile_pool(name="small", bufs=2) as small,
            tc.tile_pool(name="psumS", bufs=1, space="PSUM") as psumS,
            tc.tile_pool(name="psumSm", bufs=3, space="PSUM") as psumSm,
            tc.tile_pool(name="psumA", bufs=4, space="PSUM") as psumA,
            tc.tile_pool(name="dramp", bufs=1, space="DRAM") as dramp,
        ):
            # ---- constants ----
            ident64 = consts.tile([64, 64], F32)
            nc.gpsimd.dma_start(ident64, IDENT64.ap())
            esel = consts.tile([128, 64], F32)
            nc.gpsimd.dma_start(esel, ESEL.ap())
            f2h = consts.tile([64, 128], F32)
            nc.gpsimd.dma_start(f2h, F2H.ap())
            ones64 = consts.tile([64, 64], F32)
            nc.gpsimd.dma_start(ones64, ONES64.ap())
            i15h = consts.tile([64, 64], F16)
            nc.gpsimd.dma_start(i15h, I15H.ap())
            ih16 = consts.tile([64, 64], F16)
            nc.gpsimd.dma_start(ih16, IH16.ap())
            identb = consts.tile([128, 128], F16)
            nc.gpsimd.dma_start(identb, ID128H.ap())
            W2 = consts.tile([128, 128], F16)
            nc.gpsimd.memset(W2, 0.0)

            # ---- phase 1: load + cast(+row sums) + DMA-transpose + Gram ----
            S_ps = psumS.tile([128, 128], F32)
            srow = consts.tile([128, len(tile_geom)], F32)

            xb_tiles = []
            gi = 0
            for t, (pair, off, w) in enumerate(tile_geom):
                nch = w // 128
                xt = xpool.tile([128, w], F32, tag="xt", name=f"xt{t}",
                                padded_shape=[128, XTILE_W])
                nc.sync.dma_start(xt, Xv[pair, :, off:off + w])
                xb = xbpool.tile([128, w], F16, tag=f"xb{t}", name=f"xb{t}", bufs=1)
                if CAST_MODE == "fused":
                    nc.vector.tensor_scalar(
                        xb, xt, 1.0, None, mybir.AluOpType.mult,
                        mybir.AluOpType.add, accum_out=srow[:, t:t + 1],
                    )
                else:
                    nc.vector.tensor_copy(xb, xt)
                    nc.vector.tensor_reduce(
                        srow[:, t:t + 1], xt, axis=mybir.AxisListType.X,
                        op=mybir.AluOpType.add,
                    )
                xb_tiles.append(xb)
                xTt = xTpool.tile([128, nch, 128], F16, tag="xT", name=f"xT{t}",
                                  padded_shape=[128, 16, 128])
                if TRANSPOSE_MODE == "dma_scalar":
                    nc.scalar.dma_start(xTt, xb, transpose=True)
                elif TRANSPOSE_MODE == "dma_sync":
                    nc.sync.dma_start(xTt, xb, transpose=True)
                elif TRANSPOSE_MODE == "pe":
                    for g in range(nch):
                        tp = psumA.tile([128, 128], F16, tag="ap", name=f"tp{t}_{g}",
                                        padded_shape=[128, 1024])
                        nc.tensor.transpose(
                            tp, xb[:, g * 128:(g + 1) * 128], identb
                        )
                        nc.vector.tensor_copy(xTt[:, g, :], tp)
                else:
                    raise ValueError(TRANSPOSE_MODE)
                for k in range(nch):
                    nc.tensor.matmul(
                        S_ps,
                        xTt[:, k, :],
                        xTt[:, k, :],
                        start=(gi == 0),
                        stop=(gi == n_chunks - 1),
                        skip_group_check=True,
                    )
                    gi += 1

            if do_fold:
                # ---- local fold to (64,65) + AllGather + on-chip reduce ----
                S_sb = small.tile([128, 129], F32, tag="ssb")
                nc.vector.tensor_copy(S_sb[:, 0:128], S_ps)
                nc.vector.tensor_reduce(
                    S_sb[:, 128:129], srow, axis=mybir.AxisListType.X,
                    op=mybir.AluOpType.add,
                )
                ef_ps = psumSm.tile([64, 129], F32, tag="sm", name="ef_ps")
                nc.tensor.matmul(ef_ps, esel, S_sb, start=True, stop=True)
                pack = small.tile([64, 65], F32, tag="pack")
                nc.vector.tensor_add(pack[:, 0:64], S_sb[0:64, 0:64], ef_ps[:, 64:128])
                nc.vector.tensor_add(pack[:, 64:65], S_sb[0:64, 128:129],
                                     ef_ps[:, 128:129])
                agin = dramp.tile([64, 65], F32, tag="agin")
                agout = dramp.tile([64 * ncores, 65], F32, tag="agout")
                nc.sync.dma_start(agin, pack)
                nc.gpsimd.collective_compute(
                    "AllGather",
                    mybir.AluOpType.bypass,
                    replica_groups=[list(range(ncores))],
                    ins=[agin.opt()],
                    outs=[agout.opt()],
                )
                gath = small.tile([64, ncores, 65], F32, tag="gath")
                nc.sync.dma_start(
                    gath, agout.rearrange("(k c) n -> c k n", k=ncores)
                )
                gview = bass.AP(
                    tensor=gath.tensor,
                    offset=gath.offset,
                    ap=[gath.ap[0], [1, 65], [65, ncores]],
                )
                tot = small.tile([64, 65], F32, tag="tot")
                nc.vector.tensor_reduce(
                    tot, gview, axis=mybir.AxisListType.X, op=mybir.AluOpType.add
                )

            if do_epi:
                # ---- epilogue: mu, Sigma, trace, Newton-Schulz ----
                mu = small.tile([64, 1], F32, tag="mu")
                nc.vector.tensor_scalar_mul(mu, tot[:, 64:65], 1.0 / M_TOT)
                mur_ps = psumSm.tile([1, 64], F32, tag="sm", name="mur_ps")
                nc.tensor.matmul(mur_ps, mu, ident64, start=True, stop=True)
                mu2b_ps = psumSm.tile([128, 1], F32, tag="sm", name="mu2b_ps")
                nc.tensor.matmul(mu2b_ps, f2h, mu, start=True, stop=True)
                mu_row = small.tile([1, 64], F32, tag="murow")
                nc.vector.tensor_copy(mu_row, mur_ps)
                mu_row_negM = small.tile([1, 64], F32, tag="murown")
                nc.vector.tensor_scalar_mul(mu_row_negM, mur_ps, -float(M_TOT))
                negmu2 = consts.tile([128, 1], F32)
                nc.vector.tensor_scalar_mul(negmu2, mu2b_ps, -1.0)
                outer_ps = psumSm.tile([64, 64], F32, tag="sm", name="outer_ps")
                nc.tensor.matmul(outer_ps, mu_row_negM, mu_row, start=True, stop=True)
                Sigma = small.tile([64, 64], F32, tag="sigma")
                diagm = small.tile([64, 64], F32, tag="diagm")
                dred = small.tile([64, 1], F32, tag="dred")
                if EPI_VARIANT == "nottr":
                    nc.vector.tensor_add(diagm, tot[:, 0:64], outer_ps)
                    nc.vector.tensor_scalar_mul(Sigma, diagm, 1.0 / M_TOT)
                    nc.vector.tensor_mul(diagm, Sigma, ident64)
                    nc.vector.tensor_reduce(
                        dred, diagm, axis=mybir.AxisListType.X,
                        op=mybir.AluOpType.add,
                    )
                else:
                    dumacc = small.tile([64, 1], F32, tag="dumacc")
                    nc.vector.tensor_tensor_reduce(
                        Sigma, tot[:, 0:64], outer_ps, 1.0 / M_TOT, 0.0,
                        mybir.AluOpType.add, mybir.AluOpType.max, dumacc,
                    )
                    nc.vector.tensor_tensor_reduce(
                        diagm, Sigma, ident64, 1.0, 0.0,
                        mybir.AluOpType.mult, mybir.AluOpType.add, dred,
                    )
                tr64_ps = psumSm.tile([64, 1], F32, tag="sm", name="tr64_ps")
                nc.tensor.matmul(tr64_ps, ones64, dred, start=True, stop=True)
                trm2 = small.tile([64, 1], F32, tag="trm2")
                nc.vector.tensor_scalar_mul(trm2, tr64_ps, -2.0)
                rtr_nh = small.tile([64, 1], F32, tag="rtrnh")   # = -0.5/tr
                nc.vector.reciprocal(rtr_nh, trm2)
                srtr = small.tile([64, 1], F32, tag="srtr")      # = sqrt(1/tr)
                nc.scalar.activation(srtr, rtr_nh,
                                     func=mybir.ActivationFunctionType.Sqrt,
                                     scale=-2.0)
                Sh = small.tile([64, 64], F16, tag="sh")         # = -0.5 Sigma_N
                nc.vector.tensor_scalar_mul(Sh, Sigma, rtr_nh)
                P = small.tile([64, 64], F16, tag="P", name="P1")
                nc.vector.tensor_add(P, i15h, Sh)                # NS iteration 1

                # ---- mu pre-subtract on the fp16 shadows (overlaps NS) ----
                for t, (pair, off, w) in (
                        [] if EPI_VARIANT == "nosub" else list(enumerate(tile_geom))):
                    xb = xb_tiles[t]
                    h = w // 2
                    for half in range(2):
                        sl = xb[:, half * h:(half + 1) * h]
                        if t < 3:
                            nc.scalar.activation(
                                sl, sl,
                                func=mybir.ActivationFunctionType.Identity,
                                bias=negmu2, scale=1.0,
                            )
                        else:
                            nc.gpsimd.tensor_scalar(
                                sl, sl, negmu2, None, mybir.AluOpType.add,
                            )

                # ---- Newton-Schulz iterations 2..5 ----
                for it in range(T_NS - 1):
                    last = it == T_NS - 2
                    psAB = psumSm.tile([64, 128], F32, tag="sm", name=f"psAB{it}")
                    nc.tensor.matmul(psAB[:, 0:64], P, P, start=True, stop=True,
                                     skip_group_check=True)
                    nc.tensor.matmul(psAB[:, 64:128], P, Sh, start=True, stop=True,
                                     skip_group_check=True)
                    PQ = small.tile([64, 128], F16, tag="PQ", name=f"PQ{it}")
                    nc.vector.tensor_copy(PQ, psAB)
                    psC = psumSm.tile([64, 64], F32, tag="sm", name=f"psC{it}")
                    nc.tensor.matmul(psC, PQ[:, 0:64], PQ[:, 64:128],
                                     start=True, stop=False, skip_group_check=True)
                    nc.tensor.matmul(psC, i15h, P, start=False, stop=True,
                                     skip_group_check=True)
                    if last:
                        wmh = small.tile([64, 64], F16, tag="wmh")
                        nc.vector.tensor_scalar_mul(wmh, psC, srtr)
                    else:
                        P = small.tile([64, 64], F16, tag="P", name=f"P{it + 2}")
                        nc.vector.tensor_copy(P, psC)

                # W2 = blockdiag(wm, wm) via two identity matmuls
                W2ps = psumSm.tile([128, 64], F32, tag="sm", name="W2ps")
                nc.tensor.matmul(W2ps[0:64, :], wmh, ih16, start=True, stop=True,
                                 skip_group_check=True)
                nc.tensor.matmul(W2ps[64:128, :], wmh, ih16, start=True, stop=True,
                                 skip_group_check=True)
                nc.vector.tensor_copy(W2[0:64, 0:64], W2ps[0:64, :])
                nc.vector.tensor_copy(W2[64:128, 64:128], W2ps[64:128, :])

            if do_apply:
                # ---- phase 3: apply y = W2 @ (xb - mu) per (128,512) chunk ----
                ci = 0
                for t, (pair, off, w) in enumerate(tile_geom):
                    for cidx in range(w // 512):
                        ap_ps = psumA.tile([128, 512], F32, tag="ap",
                                           name=f"ap{t}_{cidx}",
                                           padded_shape=[128, 512])
                        nc.tensor.matmul(
                            ap_ps,
                            W2,
                            xb_tiles[t][:, cidx * 512:(cidx + 1) * 512],
                            start=True,
                            stop=True,
                        )
                        yt = ypool.tile([128, 512], F32, tag="yt",
                                        name=f"yt{t}_{cidx}")
                        if ci % 2 == 0:
                            nc.vector.tensor_copy(yt, ap_ps)
                        else:
                            nc.scalar.copy(yt, ap_ps)
                        co = off + cidx * 512
                        nc.sync.dma_start(Yv[pair, :, co:co + 512], yt)
                        ci += 1
            else:
                zt = ypool.tile([128, 512], F32, tag="yt", name="zt")
                nc.vector.memset(zt, 0.0)
                if do_epi:
                    nc.vector.tensor_add(zt[:, 0:128], zt[:, 0:128], W2)
                elif do_fold:
                    nc.vector.tensor_add(zt[0:64, 0:65], zt[0:64, 0:65], tot)
                for t, (pair, off, w) in enumerate(tile_geom):
                    for cidx in range(w // 512):
                        co = off + cidx * 512
                        nc.sync.dma_start(Yv[pair, :, co:co + 512], zt)

    nc.finalize()
    return nc


def _host_consts():
    i64 = np.eye(64, dtype=np.float32)
    esel = np.zeros((128, 64), dtype=np.float32)
    esel[64:, :] = i64
    f2h = np.concatenate([i64, i64], axis=1)
    ones64 = np.ones((64, 64), dtype=np.float32)
    i15h = (1.5 * np.eye(64)).astype(np.float16)
    ih16 = np.eye(64, dtype=np.float16)
    id128h = np.eye(128, dtype=np.float16)
    return {
        "IDENT64": i64,
        "ESEL": esel,
        "F2H": f2h,
        "ONES64": ones64,
        "I15H": i15h,
        "IH16": ih16,
        "ID128H": id128h,
    }


NCORES_RUN = NCORES


def _get_nc():
    key = f"nc{NCORES_RUN}"
    if key not in _CACHE:
        _CACHE[key] = _build_bass(NCORES_RUN)
    return _CACHE[key]


def run(X, **spmd_kwargs):
    """Run the SPMD kernel; returns (Y_full, BassKernelResults)."""
    X = np.ascontiguousarray(np.asarray(X), dtype=np.float32)
    assert X.shape == (B, C, L), X.shape
    nc = _get_nc()
    consts = _host_consts()
    n = NCORES_RUN
    in_maps = [
        {"X": X[c * BPC:(c + 1) * BPC], **consts} for c in range(n)
    ]
    res = run_bass_kernel_spmd(nc, in_maps, core_ids=list(range(n)), **spmd_kwargs)
    Y = np.concatenate([res.results[c]["Y"] for c in range(n)], axis=0)
    return Y, res


def kernel(X):
    Y, _ = run(X)
    return Y


# revision 20
# speedup vs baseline: 2.0484x; 2.0484x over previous
"""IterNorm (training-mode whitening, num_groups=1) Bass/Tile kernel for 8 trn2 cores.

Strategy (data-parallel over batch B, per sharding hint):
  - Each of the 8 cores gets 4 of the 32 batches: X_shard (4, 64, 8192) f32.
  - Batches are stacked in pairs onto 128 SBUF partitions (p0-63 = even batch
    channels, 64-127 = odd batch channels); full 128-partition HBM DMAs.
  - Stats pass: per tile, f32 load -> DVE cast to fp16 shadow -> XBAR
    DMA-transpose (fp16 SBUF->SBUF on the ACT HWDGE queue, 14ns per 16x128
    tile) producing chunked (128, nch, 130) transposed layouts with a memset
    ones column -> accumulating 128x129 fp16 PE matmul with f32 PSUM
    (cols 0..127: stacked second-moment, col 128: channel sums).  No PE
    transposes, no PSUM->SBUF copies: PE does only the Gram matmuls and the
    stats phase is HBM-load-bound.
  - Stacked block folded locally to (64,65) (selector matmul), combined
    across cores with a 16.6 KB AllGather + on-chip reduction.
  - Replicated epilogue: Sigma via one fused DVE op (tot + M*outer)*(1/M),
    trace via fused diag-mask reduce + one broadcast matmul, 1 DVE
    reciprocal; Newton-Schulz in fp16 with iteration 1 folded into
    P1 = 1.5I - 0.5 Sigma_N (DVE add) and 4 PE iterations of
    {P2|Q paired matmuls in one PSUM bank -> one DVE copy -> C matmuls}.
  - Apply pass: mu pre-subtracted from the fp16 shadows in place (ACT and
    GpSimd engines, hidden under the NS iterations), then per (128,512)
    chunk: y = W2 @ xc on PE, PSUM->SBUF copy alternating DVE/ACT, per-chunk
    f32 stores.  W2 = blockdiag(wm, wm) fp16 via SBUF->SBUF DMA.

Self-contained: hardcodes shapes and builds all constant inputs on the host.
"""

import sys

for _p in ("/opt/trn_rl_repo",):
    if _p not in sys.path:
        sys.path.insert(0, _p)

import numpy as np

import concourse.bass as bass  # noqa: F401
import concourse.tile as tile
from concourse import bacc, mybir
from concourse.bass_utils import run_bass_kernel_spmd

NCORES = 8
B, C, L = 32, 64, 8192
BPC = B // NCORES            # batches per core
M_TOT = B * L
EPS = 1e-5                   # folded away: negligible vs fp16 stats noise
T_NS = 5
F32 = mybir.dt.float32
F16 = mybir.dt.float16
XT_PITCH = 130
XTILE_W = 2048
TILE_PLAN = [2048, 2048, 2048, 1536, 512]   # per pair; sums to L
RING_CHUNKS = [8, 8, 8, 4, 0]     # per TILE_PLAN position
CAST_MODE = "fused"
STOP_AFTER = "full"
EPI_VARIANT = "nottr"   # tensor_tensor_reduce crashes on hw

_CACHE = {}


def _build_bass(ncores=NCORES):
    nc = bacc.Bacc("TRN2", target_bir_lowering=False, debug=False, num_devices=ncores)

    X = nc.dram_tensor("X", [BPC, C, L], F32, kind="ExternalInput")
    Y = nc.dram_tensor("Y", [BPC, C, L], F32, kind="ExternalOutput")
    IDENT64 = nc.dram_tensor("IDENT64", [64, 64], F32, kind="ExternalInput")
    ESEL = nc.dram_tensor("ESEL", [128, 64], F32, kind="ExternalInput")
    F2H = nc.dram_tensor("F2H", [64, 128], F32, kind="ExternalInput")
    ONES64 = nc.dram_tensor("ONES64", [64, 64], F32, kind="ExternalInput")
    I15H = nc.dram_tensor("I15H", [64, 64], F16, kind="ExternalInput")
    IH16 = nc.dram_tensor("IH16", [64, 64], F16, kind="ExternalInput")
    ID128H = nc.dram_tensor("ID128H", [128, 128], F16, kind="ExternalInput")

    Xv = X.ap().rearrange("(p i) c l -> p (i c) l", i=2)
    Yv = Y.ap().rearrange("(p i) c l -> p (i c) l", i=2)
    tile_geom = []
    for pair in range(2):
        off = 0
        for w in TILE_PLAN:
            tile_geom.append((pair, off, w))
            off += w
    n_chunks = 2 * L // 128
    do_fold = STOP_AFTER in ("collective", "epilogue", "full")
    do_epi = STOP_AFTER in ("epilogue", "full")
    do_apply = STOP_AFTER == "full"

    with tile.TileContext(nc) as tc:
        with (
            tc.tile_pool(name="consts", bufs=1) as consts,
            tc.tile_pool(name="xpool", bufs=3) as xpool,
            tc.tile_pool(name="xTpool", bufs=3) as xTpool,
            tc.tile_pool(name="xbpool", bufs=1) as xbpool,
            tc.tile_pool(name="ypool", bufs=6) as ypool,
            tc.tile_pool(name="small", bufs=2) as small,
            tc.tile_pool(name="psumS", bufs=1, space="PSUM") as psumS,
            tc.tile_pool(name="psumSm", bufs=3, space="PSUM") as psumSm,
            tc.tile_pool(name="psumA", bufs=4, space="PSUM") as psumA,
            tc.tile_pool(name="dramp", bufs=1, space="DRAM") as dramp,
        ):
            # ---- constants ----
            ident64 = consts.tile([64, 64], F32)
            nc.gpsimd.dma_start(ident64, IDENT64.ap())
            esel = consts.tile([128, 64], F32)
            nc.gpsimd.dma_start(esel, ESEL.ap())
            f2h = consts.tile([64, 128], F32)
            nc.gpsimd.dma_start(f2h, F2H.ap())
            ones64 = consts.tile([64, 64], F32)
            nc.gpsimd.dma_start(ones64, ONES64.ap())
            i15h = consts.tile([64, 64], F16)
            nc.gpsimd.dma_start(i15h, I15H.ap())
            ih16 = consts.tile([64, 64], F16)
            nc.gpsimd.dma_start(ih16, IH16.ap())
            identb = consts.tile([128, 128], F16)
            nc.gpsimd.dma_start(identb, ID128H.ap())
            W2 = consts.tile([128, 128], F16)
            nc.gpsimd.memset(W2, 0.0)

            # ---- phase 1: load + cast(+row sums) + hybrid transpose + Gram ----
            # XBAR DMA-transpose throughput is ~660ns per 128-col chunk per
            # HWDGE ring (descriptor-issue bound), so the PE transposes most
            # chunks (groups of 4 into one PSUM bank + a single copy) and the
            # two rings take wide spans of the remainder.
            S_ps = psumS.tile([128, 128], F32)
            srow = consts.tile([128, 2 * len(tile_geom)], F32)

            xb_tiles = []
            gi = 0
            cpi = 0
            for t, (pair, off, w) in enumerate(tile_geom):
                nch = w // 128
                xt = xpool.tile([128, w], F32, tag="xt", name=f"xt{t}",
                                padded_shape=[128, XTILE_W])
                nc.sync.dma_start(xt, Xv[pair, :, off:off + w])
                # fp16 shadow; casts split DVE/ACT, both with fused row sums
                xb = xbpool.tile([128, w], F16, tag=f"xb{t}", name=f"xb{t}", bufs=1)
                h = w // 2
                nc.vector.tensor_scalar(
                    xb[:, 0:h], xt[:, 0:h], 1.0, None, mybir.AluOpType.mult,
                    mybir.AluOpType.add, accum_out=srow[:, 2 * t:2 * t + 1],
                )
                nc.scalar.activation(
                    xb[:, h:w], xt[:, h:w],
                    func=mybir.ActivationFunctionType.Identity,
                    accum_out=srow[:, 2 * t + 1:2 * t + 2],
                )
                xb_tiles.append(xb)
                xTt = xTpool.tile([128, nch, 128], F16, tag="xT", name=f"xT{t}",
                                  padded_shape=[128, 16, 128])
                # ring spans: last RING_CHUNKS[t] chunks, split across the
                # sync and scalar HWDGE rings
                nring = RING_CHUNKS[min(t % len(TILE_PLAN), len(RING_CHUNKS) - 1)]
                npe = nch - nring
                if nring:
                    half = nring // 2
                    s0 = npe
                    if half:
                        nc.sync.dma_start(
                            xTt[:, s0:s0 + half, :],
                            xb[:, s0 * 128:(s0 + half) * 128], transpose=True)
                    nc.scalar.dma_start(
                        xTt[:, s0 + half:nch, :],
                        xb[:, (s0 + half) * 128:nch * 128], transpose=True)
                # PE groups of 4 chunks -> one PSUM bank -> one copy
                for g0 in range(0, npe, 4):
                    gn = min(4, npe - g0)
                    tp = psumA.tile([128, gn * 128], F16, tag="ap",
                                    name=f"tp{t}_{g0}", padded_shape=[128, 1024])
                    for k in range(gn):
                        nc.tensor.transpose(
                            tp[:, k * 128:(k + 1) * 128],
                            xb[:, (g0 + k) * 128:(g0 + k + 1) * 128], identb)
                    cp = tp.rearrange("p (a b) -> p a b", a=gn)
                    if cpi % 3 == 0:
                        nc.vector.tensor_copy(xTt[:, g0:g0 + gn, :], cp)
                    else:
                        nc.scalar.copy(xTt[:, g0:g0 + gn, :], cp)
                    cpi += 1
                for k in range(nch):
                    nc.tensor.matmul(
                        S_ps,
                        xTt[:, k, :],
                        xTt[:, k, :],
                        start=(gi == 0),
                        stop=(gi == n_chunks - 1),
                        skip_group_check=True,
                    )
                    gi += 1

            if do_fold:
                # ---- local fold to (64,65) + AllGather + on-chip reduce ----
                S_sb = small.tile([128, 129], F32, tag="ssb")
                nc.vector.tensor_copy(S_sb[:, 0:128], S_ps)
                nc.vector.tensor_reduce(
                    S_sb[:, 128:129], srow, axis=mybir.AxisListType.X,
                    op=mybir.AluOpType.add,
                )
                ef_ps = psumSm.tile([64, 129], F32, tag="sm", name="ef_ps")
                nc.tensor.matmul(ef_ps, esel, S_sb, start=True, stop=True)
                pack = small.tile([64, 65], F32, tag="pack")
                nc.vector.tensor_add(pack[:, 0:64], S_sb[0:64, 0:64], ef_ps[:, 64:128])
                nc.vector.tensor_add(pack[:, 64:65], S_sb[0:64, 128:129],
                                     ef_ps[:, 128:129])
                agin = dramp.tile([64, 65], F32, tag="agin")
                agout = dramp.tile([64 * ncores, 65], F32, tag="agout")
                nc.sync.dma_start(agin, pack)
                nc.gpsimd.collective_compute(
                    "AllGather",
                    mybir.AluOpType.bypass,
                    replica_groups=[list(range(ncores))],
                    ins=[agin.opt()],
                    outs=[agout.opt()],
                )
                gath = small.tile([64, ncores, 65], F32, tag="gath")
                nc.sync.dma_start(
                    gath, agout.rearrange("(k c) n -> c k n", k=ncores)
                )
                gview = bass.AP(
                    tensor=gath.tensor,
                    offset=gath.offset,
                    ap=[gath.ap[0], [1, 65], [65, ncores]],
                )
                tot = small.tile([64, 65], F32, tag="tot")
                nc.vector.tensor_reduce(
                    tot, gview, axis=mybir.AxisListType.X, op=mybir.AluOpType.add
                )

            if do_epi:
                # ---- epilogue: mu, Sigma, trace, Newton-Schulz ----
                mu = small.tile([64, 1], F32, tag="mu")
                nc.vector.tensor_scalar_mul(mu, tot[:, 64:65], 1.0 / M_TOT)
                mur_ps = psumSm.tile([1, 64], F32, tag="sm", name="mur_ps")
                nc.tensor.matmul(mur_ps, mu, ident64, start=True, stop=True)
                mu2b_ps = psumSm.tile([128, 1], F32, tag="sm", name="mu2b_ps")
                nc.tensor.matmul(mu2b_ps, f2h, mu, start=True, stop=True)
                mu_row = small.tile([1, 64], F32, tag="murow")
                nc.vector.tensor_copy(mu_row, mur_ps)
                mu_row_negM = small.tile([1, 64], F32, tag="murown")
                nc.vector.tensor_scalar_mul(mu_row_negM, mur_ps, -float(M_TOT))
                negmu2 = consts.tile([128, 1], F32)
                nc.vector.tensor_scalar_mul(negmu2, mu2b_ps, -1.0)
                outer_ps = psumSm.tile([64, 64], F32, tag="sm", name="outer_ps")
                nc.tensor.matmul(outer_ps, mu_row_negM, mu_row, start=True, stop=True)
                Sigma = small.tile([64, 64], F32, tag="sigma")
                diagm = small.tile([64, 64], F32, tag="diagm")
                dred = small.tile([64, 1], F32, tag="dred")
                if EPI_VARIANT == "nottr":
                    nc.vector.tensor_add(diagm, tot[:, 0:64], outer_ps)
                    nc.vector.tensor_scalar_mul(Sigma, diagm, 1.0 / M_TOT)
                    nc.vector.tensor_mul(diagm, Sigma, ident64)
                    nc.vector.tensor_reduce(
                        dred, diagm, axis=mybir.AxisListType.X,
                        op=mybir.AluOpType.add,
                    )
                else:
                    dumacc = small.tile([64, 1], F32, tag="dumacc")
                    nc.vector.tensor_tensor_reduce(
                        Sigma, tot[:, 0:64], outer_ps, 1.0 / M_TOT, 0.0,
                        mybir.AluOpType.add, mybir.AluOpType.max, dumacc,
                    )
                    nc.vector.tensor_tensor_reduce(
                        diagm, Sigma, ident64, 1.0, 0.0,
                        mybir.AluOpType.mult, mybir.AluOpType.add, dred,
                    )
                tr64_ps = psumSm.tile([64, 1], F32, tag="sm", name="tr64_ps")
                nc.tensor.matmul(tr64_ps, ones64, dred, start=True, stop=True)
                trm2 = small.tile([64, 1], F32, tag="trm2")
                nc.vector.tensor_scalar_mul(trm2, tr64_ps, -2.0)
                rtr_nh = small.tile([64, 1], F32, tag="rtrnh")   # = -0.5/tr
                nc.vector.reciprocal(rtr_nh, trm2)
                srtr = small.tile([64, 1], F32, tag="srtr")      # = sqrt(1/tr)
                nc.scalar.activation(srtr, rtr_nh,
                                     func=mybir.ActivationFunctionType.Sqrt,
                                     scale=-2.0)
                Sh = small.tile([64, 64], F16, tag="sh")         # = -0.5 Sigma_N
                nc.vector.tensor_scalar_mul(Sh, Sigma, rtr_nh)
                P = small.tile([64, 64], F16, tag="P", name="P1")
                nc.vector.tensor_add(P, i15h, Sh)                # NS iteration 1

                # ---- mu pre-subtract, tiles 0-4 on ACT (overlaps NS) ----
                for t in range(5):
                    xb = xb_tiles[t]
                    w = tile_geom[t][2]
                    h = w // 2
                    for half in range(2):
                        sl = xb[:, half * h:(half + 1) * h]
                        nc.scalar.activation(
                            sl, sl,
                            func=mybir.ActivationFunctionType.Identity,
                            bias=negmu2, scale=1.0,
                        )

                # ---- Newton-Schulz iterations 2..5 ----
                for it in range(T_NS - 1):
                    last = it == T_NS - 2
                    psAB = psumSm.tile([64, 128], F32, tag="sm", name=f"psAB{it}")
                    nc.tensor.matmul(psAB[:, 0:64], P, P, start=True, stop=True,
                                     skip_group_check=True)
                    nc.tensor.matmul(psAB[:, 64:128], P, Sh, start=True, stop=True,
                                     skip_group_check=True)
                    PQ = small.tile([64, 128], F16, tag="PQ", name=f"PQ{it}")
                    nc.vector.tensor_copy(PQ, psAB)
                    psC = psumSm.tile([64, 64], F32, tag="sm", name=f"psC{it}")
                    nc.tensor.matmul(psC, PQ[:, 0:64], PQ[:, 64:128],
                                     start=True, stop=False, skip_group_check=True)
                    nc.tensor.matmul(psC, i15h, P, start=False, stop=True,
                                     skip_group_check=True)
                    if last:
                        wmh = small.tile([64, 64], F16, tag="wmh")
                        nc.vector.tensor_scalar_mul(wmh, psC, srtr)
                    else:
                        P = small.tile([64, 64], F16, tag="P", name=f"P{it + 2}")
                        nc.vector.tensor_copy(P, psC)

                # W2 = blockdiag(wm, wm) via two identity matmuls
                W2ps = psumSm.tile([128, 64], F32, tag="sm", name="W2ps")
                nc.tensor.matmul(W2ps[0:64, :], wmh, ih16, start=True, stop=True,
                                 skip_group_check=True)
                nc.tensor.matmul(W2ps[64:128, :], wmh, ih16, start=True, stop=True,
                                 skip_group_check=True)
                nc.vector.tensor_copy(W2[0:64, 0:64], W2ps[0:64, :])
                nc.vector.tensor_copy(W2[64:128, 64:128], W2ps[64:128, :])

                # ---- mu pre-subtract, tiles 5-9 on DVE (fast fp16) ----
                for t in range(5, len(tile_geom)):
                    xb = xb_tiles[t]
                    w = tile_geom[t][2]
                    h = w // 2
                    for half in range(2):
                        sl = xb[:, half * h:(half + 1) * h]
                        nc.vector.tensor_scalar(
                            sl, sl, negmu2, None, mybir.AluOpType.add,
                        )

            if do_apply:
                # ---- phase 3: apply y = W2 @ (xb - mu) per (128,512) chunk ----
                ci = 0
                for t, (pair, off, w) in enumerate(tile_geom):
                    for cidx in range(w // 512):
                        ap_ps = psumA.tile([128, 512], F32, tag="ap",
                                           name=f"ap{t}_{cidx}",
                                           padded_shape=[128, 512])
                        nc.tensor.matmul(
                            ap_ps,
                            W2,
                            xb_tiles[t][:, cidx * 512:(cidx + 1) * 512],
                            start=True,
                            stop=True,
                        )
                        yt = ypool.tile([128, 512], F32, tag="yt",
                                        name=f"yt{t}_{cidx}")
                        if ci % 2 == 0:
                            nc.vector.tensor_copy(yt, ap_ps)
                        else:
                            nc.scalar.copy(yt, ap_ps)
                        co = off + cidx * 512
                        nc.sync.dma_start(Yv[pair, :, co:co + 512], yt)
                        ci += 1
            else:
                zt = ypool.tile([128, 512], F32, tag="yt", name="zt")
                nc.vector.memset(zt, 0.0)
                if do_epi:
                    nc.vector.tensor_add(zt[:, 0:128], zt[:, 0:128], W2)
                elif do_fold:
                    nc.vector.tensor_add(zt[0:64, 0:65], zt[0:64, 0:65], tot)
                for t, (pair, off, w) in enumerate(tile_geom):
                    for cidx in range(w // 512):
                        co = off + cidx * 512
                        nc.sync.dma_start(Yv[pair, :, co:co + 512], zt)

    nc.finalize()
    return nc


def _host_consts():
    i64 = np.eye(64, dtype=np.float32)
    esel = np.zeros((128, 64), dtype=np.float32)
    esel[64:, :] = i64
    f2h = np.concatenate([i64, i64], axis=1)
    ones64 = np.ones((64, 64), dtype=np.float32)
    i15h = (1.5 * np.eye(64)).astype(np.float16)
    ih16 = np.eye(64, dtype=np.float16)
    id128h = np.eye(128, dtype=np.float16)
    return {
        "IDENT64": i64,
        "ESEL": esel,
        "F2H": f2h,
        "ONES64": ones64,
        "I15H": i15h,
        "IH16": ih16,
        "ID128H": id128h,
    }


NCORES_RUN = NCORES


def _get_nc():
    key = f"nc{NCORES_RUN}"
    if key not in _CACHE:
        _CACHE[key] = _build_bass(NCORES_RUN)
    return _CACHE[key]


def run(X, **spmd_kwargs):
    """Run the SPMD kernel; returns (Y_full, BassKernelResults)."""
    X = np.ascontiguousarray(np.asarray(X), dtype=np.float32)
    assert X.shape == (B, C, L), X.shape
    nc = _get_nc()
    consts = _host_consts()
    n = NCORES_RUN
    in_maps = [
        {"X": X[c * BPC:(c + 1) * BPC], **consts} for c in range(n)
    ]
    res = run_bass_kernel_spmd(nc, in_maps, core_ids=list(range(n)), **spmd_kwargs)
    Y = np.concatenate([res.results[c]["Y"] for c in range(n)], axis=0)
    return Y, res


def kernel(X):
    Y, _ = run(X)
    return Y


# revision 21
# speedup vs baseline: 2.5815x; 1.2603x over previous
"""IterNorm (training-mode whitening, num_groups=1) Bass/Tile kernel for 8 trn2 cores.

Strategy (data-parallel over batch B, per sharding hint):
  - Each of the 8 cores gets 4 of the 32 batches: X_shard (4, 64, 8192) f32.
  - Batches are stacked in pairs onto 128 SBUF partitions (p0-63 = even batch
    channels, 64-127 = odd batch channels); full 128-partition HBM DMAs.
  - Stats pass, pipelined per tile: f32 load -> cast to an fp16 shadow split
    DVE/ACT with the per-channel row sums fused in via accum_out -> PE
    transposes in groups of 4 chunks into one PSUM bank -> one DVE copy per
    group -> accumulating 128x128 fp16 Gram matmul into f32 PSUM.  PE does
    ~163ns per 128-col chunk (transpose + matmul, weight loads overlapped),
    so the phase tracks the HBM load roofline.
  - The stacked (128,128) block + sums are folded locally to (64,65)
    (selector matmul) and combined across cores with a 16.6 KB AllGather +
    on-chip reduction.
  - Replicated epilogue: Sigma/trace with the DVE kept clear of bulk work,
    trace broadcast via one all-ones matmul; Newton-Schulz in fp16 with
    iteration 1 folded into P1 = 1.5I - 0.5 Sigma_N and 4 PE iterations of
    {P2|Q paired matmuls in one PSUM bank -> one DVE cast -> C matmuls}.
    W2 = blockdiag(wm, wm) built with two identity matmuls (PE can cross
    partitions; DVE cannot).
  - Apply pass: mu pre-subtracted from the fp16 shadows in place on ACT
    (hidden under the NS iterations), then per (128,512) chunk:
    y = W2 @ xc on PE -> PSUM->SBUF copy (DVE, ACT helps on the tail) ->
    per-chunk f32 store.

Notes vs. hardware: tensor_tensor_reduce crashes on hw (sim-only); GpSimd
ALU ops run ~10 G elem/s; the XBAR DMA-transpose ucode is descriptor-bound
(~1us per 128-col chunk per ring) -- all three are avoided.

Self-contained: hardcodes shapes and builds all constant inputs on the host.
"""

import sys

for _p in ("/opt/trn_rl_repo",):
    if _p not in sys.path:
        sys.path.insert(0, _p)

import numpy as np

import concourse.bass as bass  # noqa: F401
import concourse.tile as tile
from concourse import bacc, mybir
from concourse.bass_utils import run_bass_kernel_spmd

NCORES = 8
B, C, L = 32, 64, 8192
BPC = B // NCORES            # batches per core
M_TOT = B * L
T_NS = 5
F32 = mybir.dt.float32
F16 = mybir.dt.float16
XTILE_W = 2048
TILE_PLAN = [2048, 2048, 2048, 1536, 512]   # per pair; sums to L

_CACHE = {}


def _build_bass(ncores=NCORES):
    nc = bacc.Bacc("TRN2", target_bir_lowering=False, debug=False, num_devices=ncores)

    X = nc.dram_tensor("X", [BPC, C, L], F32, kind="ExternalInput")
    Y = nc.dram_tensor("Y", [BPC, C, L], F32, kind="ExternalOutput")
    # packed constants: one f32 and one f16 tensor (2 DMAs)
    CF32 = nc.dram_tensor("CF32", [128, 320], F32, kind="ExternalInput")
    CF16 = nc.dram_tensor("CF16", [128, 256], F16, kind="ExternalInput")

    Xv = X.ap().rearrange("(p i) c l -> p (i c) l", i=2)
    Yv = Y.ap().rearrange("(p i) c l -> p (i c) l", i=2)
    tile_geom = []
    for pair in range(2):
        off = 0
        for w in TILE_PLAN:
            tile_geom.append((pair, off, w))
            off += w
    n_chunks = 2 * L // 128
    ntiles = len(tile_geom)

    with tile.TileContext(nc) as tc:
        with (
            tc.tile_pool(name="consts", bufs=1) as consts,
            tc.tile_pool(name="xpool", bufs=3) as xpool,
            tc.tile_pool(name="xTpool", bufs=3) as xTpool,
            tc.tile_pool(name="xbpool", bufs=1) as xbpool,
            tc.tile_pool(name="ypool", bufs=6) as ypool,
            tc.tile_pool(name="small", bufs=2) as small,
            tc.tile_pool(name="psumS", bufs=1, space="PSUM") as psumS,
            tc.tile_pool(name="psumSm", bufs=3, space="PSUM") as psumSm,
            tc.tile_pool(name="psumA", bufs=4, space="PSUM") as psumA,
            tc.tile_pool(name="dramp", bufs=1, space="DRAM") as dramp,
        ):
            # ---- constants (packed: 2 DMAs) ----
            cf32 = consts.tile([128, 320], F32)
            nc.gpsimd.dma_start(cf32, CF32.ap())
            cf16 = consts.tile([128, 256], F16)
            nc.gpsimd.dma_start(cf16, CF16.ap())
            esel = cf32[:, 0:64]            # (128,64) rows 64:128 = I64
            ident64 = cf32[0:64, 64:128]    # (64,64) I
            f2h = cf32[0:64, 128:256]       # (64,128) [I|I]
            ones64 = cf32[0:64, 256:320]    # (64,64) ones
            i15h = cf16[0:64, 0:64]         # (64,64) 1.5 I fp16
            ih16 = cf16[0:64, 64:128]       # (64,64) I fp16
            identb = cf16[:, 128:256]       # (128,128) I fp16
            W2 = consts.tile([128, 128], F16)
            nc.gpsimd.memset(W2, 0.0)

            # ---- phase 1: load + cast(+row sums) + PE transpose + Gram ----
            S_ps = psumS.tile([128, 128], F32)
            srow = consts.tile([128, 2 * ntiles], F32)

            xb_tiles = []
            gi = 0
            for t, (pair, off, w) in enumerate(tile_geom):
                nch = w // 128
                xt = xpool.tile([128, w], F32, tag="xt", name=f"xt{t}",
                                padded_shape=[128, XTILE_W])
                nc.sync.dma_start(xt, Xv[pair, :, off:off + w])
                # fp16 shadow; cast split 40% DVE / 60% ACT, row sums fused
                xb = xbpool.tile([128, w], F16, tag=f"xb{t}", name=f"xb{t}", bufs=1)
                h = (2 * nch // 5) * 128
                nc.vector.tensor_scalar(
                    xb[:, 0:h], xt[:, 0:h], 1.0, None, mybir.AluOpType.mult,
                    mybir.AluOpType.add, accum_out=srow[:, 2 * t:2 * t + 1],
                )
                nc.scalar.activation(
                    xb[:, h:w], xt[:, h:w],
                    func=mybir.ActivationFunctionType.Identity,
                    accum_out=srow[:, 2 * t + 1:2 * t + 2],
                )
                xb_tiles.append(xb)
                xTt = xTpool.tile([128, nch, 128], F16, tag="xT", name=f"xT{t}",
                                  padded_shape=[128, 16, 128])
                # PE transposes, groups of 4 chunks -> one PSUM bank -> 1 copy
                for g0 in range(0, nch, 4):
                    gn = min(4, nch - g0)
                    tp = psumA.tile([128, gn * 128], F16, tag="ap",
                                    name=f"tp{t}_{g0}", padded_shape=[128, 1024])
                    for k in range(gn):
                        nc.tensor.transpose(
                            tp[:, k * 128:(k + 1) * 128],
                            xb[:, (g0 + k) * 128:(g0 + k + 1) * 128], identb)
                    cp = tp.rearrange("p (a b) -> p a b", a=gn)
                    nc.vector.tensor_copy(xTt[:, g0:g0 + gn, :], cp)
                for k in range(nch):
                    nc.tensor.matmul(
                        S_ps,
                        xTt[:, k, :],
                        xTt[:, k, :],
                        start=(gi == 0),
                        stop=(gi == n_chunks - 1),
                        skip_group_check=True,
                    )
                    gi += 1

            # ---- local fold to (64,65) + AllGather + on-chip reduce ----
            S_sb = small.tile([128, 129], F32, tag="ssb")
            nc.vector.tensor_copy(S_sb[:, 0:128], S_ps)
            nc.vector.tensor_reduce(
                S_sb[:, 128:129], srow, axis=mybir.AxisListType.X,
                op=mybir.AluOpType.add,
            )
            ef_ps = psumSm.tile([64, 129], F32, tag="sm", name="ef_ps")
            nc.tensor.matmul(ef_ps, esel, S_sb, start=True, stop=True)
            pack = small.tile([64, 65], F32, tag="pack")
            nc.vector.tensor_add(pack[:, 0:64], S_sb[0:64, 0:64], ef_ps[:, 64:128])
            nc.vector.tensor_add(pack[:, 64:65], S_sb[0:64, 128:129],
                                 ef_ps[:, 128:129])
            agin = dramp.tile([64, 65], F32, tag="agin")
            agout = dramp.tile([64 * ncores, 65], F32, tag="agout")
            nc.sync.dma_start(agin, pack)
            nc.gpsimd.collective_compute(
                "AllGather",
                mybir.AluOpType.bypass,
                replica_groups=[list(range(ncores))],
                ins=[agin.opt()],
                outs=[agout.opt()],
            )
            gath = small.tile([64, ncores, 65], F32, tag="gath")
            nc.sync.dma_start(
                gath, agout.rearrange("(k c) n -> c k n", k=ncores)
            )
            # reduce the per-core blocks: view (64, k, 65) as (64, 65, k)
            gview = bass.AP(
                tensor=gath.tensor,
                offset=gath.offset,
                ap=[gath.ap[0], [1, 65], [65, ncores]],
            )
            tot = small.tile([64, 65], F32, tag="tot")
            nc.vector.tensor_reduce(
                tot, gview, axis=mybir.AxisListType.X, op=mybir.AluOpType.add
            )

            # ---- epilogue: mu, Sigma, trace, Newton-Schulz (replicated) ----
            mu = small.tile([64, 1], F32, tag="mu")
            nc.vector.tensor_scalar_mul(mu, tot[:, 64:65], 1.0 / M_TOT)
            mur_ps = psumSm.tile([1, 64], F32, tag="sm", name="mur_ps")
            nc.tensor.matmul(mur_ps, mu, ident64, start=True, stop=True)
            mu2b_ps = psumSm.tile([128, 1], F32, tag="sm", name="mu2b_ps")
            nc.tensor.matmul(mu2b_ps, f2h, mu, start=True, stop=True)
            mu_row = small.tile([1, 64], F32, tag="murow")
            nc.vector.tensor_copy(mu_row, mur_ps)
            mu_row_negM = small.tile([1, 64], F32, tag="murown")
            nc.vector.tensor_scalar_mul(mu_row_negM, mur_ps, -float(M_TOT))
            negmu2 = consts.tile([128, 1], F32)
            nc.vector.tensor_scalar_mul(negmu2, mu2b_ps, -1.0)
            outer_ps = psumSm.tile([64, 64], F32, tag="sm", name="outer_ps")
            nc.tensor.matmul(outer_ps, mu_row_negM, mu_row, start=True, stop=True)
            Sigma = small.tile([64, 64], F32, tag="sigma")
            diagm = small.tile([64, 64], F32, tag="diagm")
            dred = small.tile([64, 1], F32, tag="dred")
            nc.vector.tensor_add(diagm, tot[:, 0:64], outer_ps)
            nc.vector.tensor_scalar_mul(Sigma, diagm, 1.0 / M_TOT)
            nc.vector.tensor_mul(diagm, Sigma, ident64)
            nc.vector.tensor_reduce(
                dred, diagm, axis=mybir.AxisListType.X, op=mybir.AluOpType.add
            )
            tr64_ps = psumSm.tile([64, 1], F32, tag="sm", name="tr64_ps")
            nc.tensor.matmul(tr64_ps, ones64, dred, start=True, stop=True)
            trm2 = small.tile([64, 1], F32, tag="trm2")
            nc.vector.tensor_scalar_mul(trm2, tr64_ps, -2.0)
            rtr_nh = small.tile([64, 1], F32, tag="rtrnh")   # = -0.5/tr
            nc.vector.reciprocal(rtr_nh, trm2)
            srtr = small.tile([64, 1], F32, tag="srtr")      # = sqrt(1/tr)
            nc.scalar.activation(srtr, rtr_nh,
                                 func=mybir.ActivationFunctionType.Sqrt,
                                 scale=-2.0)
            Sh = small.tile([64, 64], F16, tag="sh")         # = -0.5 Sigma_N
            nc.vector.tensor_scalar_mul(Sh, Sigma, rtr_nh)
            P = small.tile([64, 64], F16, tag="P", name="P1")
            nc.vector.tensor_add(P, i15h, Sh)                # NS iteration 1

            # ---- mu pre-subtract on ACT, in apply order (overlaps NS) ----
            for t in range(ntiles):
                xb = xb_tiles[t]
                w = tile_geom[t][2]
                h = w // 2
                for half in range(2):
                    sl = xb[:, half * h:(half + 1) * h]
                    nc.scalar.activation(
                        sl, sl,
                        func=mybir.ActivationFunctionType.Identity,
                        bias=negmu2, scale=1.0,
                    )

            # ---- Newton-Schulz iterations 2..5 (fp16, paired PSUM mms) ----
            for it in range(T_NS - 1):
                last = it == T_NS - 2
                psAB = psumSm.tile([64, 128], F32, tag="sm", name=f"psAB{it}")
                nc.tensor.matmul(psAB[:, 0:64], P, P, start=True, stop=True,
                                 skip_group_check=True)
                nc.tensor.matmul(psAB[:, 64:128], P, Sh, start=True, stop=True,
                                 skip_group_check=True)
                PQ = small.tile([64, 128], F16, tag="PQ", name=f"PQ{it}")
                nc.vector.tensor_copy(PQ, psAB)
                psC = psumSm.tile([64, 64], F32, tag="sm", name=f"psC{it}")
                nc.tensor.matmul(psC, PQ[:, 0:64], PQ[:, 64:128],
                                 start=True, stop=False, skip_group_check=True)
                nc.tensor.matmul(psC, i15h, P, start=False, stop=True,
                                 skip_group_check=True)
                if last:
                    wmh = small.tile([64, 64], F16, tag="wmh")
                    nc.vector.tensor_scalar_mul(wmh, psC, srtr)
                else:
                    P = small.tile([64, 64], F16, tag="P", name=f"P{it + 2}")
                    nc.vector.tensor_copy(P, psC)

            # W2 = blockdiag(wm, wm): two identity matmuls (PE crosses
            # partitions; DVE cannot)
            W2ps = psumSm.tile([128, 64], F32, tag="sm", name="W2ps")
            nc.tensor.matmul(W2ps[0:64, :], wmh, ih16, start=True, stop=True,
                             skip_group_check=True)
            nc.tensor.matmul(W2ps[64:128, :], wmh, ih16, start=True, stop=True,
                             skip_group_check=True)
            nc.vector.tensor_copy(W2[0:64, 0:64], W2ps[0:64, :])
            nc.vector.tensor_copy(W2[64:128, 64:128], W2ps[64:128, :])

            # ---- phase 3: apply y = W2 @ (xb - mu) per (128,512) chunk ----
            ci = 0
            for t, (pair, off, w) in enumerate(tile_geom):
                for cidx in range(w // 512):
                    ap_ps = psumA.tile([128, 512], F32, tag="ap",
                                       name=f"ap{t}_{cidx}",
                                       padded_shape=[128, 512])
                    nc.tensor.matmul(
                        ap_ps,
                        W2,
                        xb_tiles[t][:, cidx * 512:(cidx + 1) * 512],
                        start=True,
                        stop=True,
                    )
                    yt = ypool.tile([128, 512], F32, tag="yt",
                                    name=f"yt{t}_{cidx}")
                    if ci < 16 or ci % 2 == 0:
                        nc.vector.tensor_copy(yt, ap_ps)
                    else:
                        nc.scalar.copy(yt, ap_ps)
                    co = off + cidx * 512
                    nc.sync.dma_start(Yv[pair, :, co:co + 512], yt)
                    ci += 1

    nc.finalize()
    return nc


def _host_consts():
    i64 = np.eye(64, dtype=np.float32)
    cf32 = np.zeros((128, 320), dtype=np.float32)
    cf32[64:128, 0:64] = i64                          # esel
    cf32[0:64, 64:128] = i64                          # ident64
    cf32[0:64, 128:192] = i64                         # f2h left
    cf32[0:64, 192:256] = i64                         # f2h right
    cf32[0:64, 256:320] = 1.0                         # ones64
    cf16 = np.zeros((128, 256), dtype=np.float16)
    cf16[0:64, 0:64] = (1.5 * i64).astype(np.float16)  # i15h
    cf16[0:64, 64:128] = i64.astype(np.float16)        # ih16
    cf16[:, 128:256] = np.eye(128, dtype=np.float16)   # identb
    return {"CF32": cf32, "CF16": cf16}


NCORES_RUN = NCORES


def _get_nc():
    key = f"nc{NCORES_RUN}"
    if key not in _CACHE:
        _CACHE[key] = _build_bass(NCORES_RUN)
    return _CACHE[key]


def run(X, **spmd_kwargs):
    """Run the SPMD kernel; returns (Y_full, BassKernelResults)."""
    X = np.ascontiguousarray(np.asarray(X), dtype=np.float32)
    assert X.shape == (B, C, L), X.shape
    nc = _get_nc()
    consts = _host_consts()
    n = NCORES_RUN
    in_maps = [
        {"X": X[c * BPC:(c + 1) * BPC], **consts} for c in range(n)
    ]
    res = run_bass_kernel_spmd(nc, in_maps, core_ids=list(range(n)), **spmd_kwargs)
    Y = np.concatenate([res.results[c]["Y"] for c in range(n)], axis=0)
    return Y, res


def kernel(X):
    Y, _ = run(X)
    return Y
